# revision 3
# baseline (speedup 1.0000x reference)
"""AttMatrixCov loss kernel for 8 Trainium2 NeuronCores (raw Bass).

Math (same as the earlier Tile version): with A_t = S_t S_t^T the
pairwise sums collapse to
    sum_{i<j} |S_i^T S_j|_F^2 = 1/2 (|M|_F^2 - sum_t |A_t|_F^2),
    M = sum_t A_t,
plus exact O(N^2) host glue for the R-term and the channel branch.
Sharding: 8 cores = (natt=4) x (H row-block m=0,1); each core loads all
8 temps of S^T as fp8 [128p, 8t, 2g, 256h] and computes its 128-row
block of every A_t and of M.

The device program is hand-scheduled raw Bass (no TileContext - its
entry/exit barriers and scheduling slack cost ~5us here):

  sync:   4x input DMA (2 temps each, pipelined on one HWDGE ring,
          one semaphore per DMA since per-engine sem increments from
          different transfers interleave) -> stats out DMA. No final
          osem wait: the 2KB output lands during the multi-us exit
          sequence, long before the runtime fetches outputs.
  tensor: 8x fp8 DoubleRow matmul, each gated only on its chunk's DMA
          semaphore; runs the stream without interruption.
  vector: M = sum_t A_t folded pair-wise [128,512] as pairs complete
          (max one PSUM operand per DVE op), then fold + |M|^2
          mul/reduce while ACT squares.
  scalar: Square+accumulate the A blocks in two 2-bank quads, then
          quad1 chained behind quad0.
  gpsimd: zero-fills the pad column of the out tile.

Empirical device constraints honored here (each violation hangs the
core): ACT may only touch PSUM after the PE is completely done
(regardless of bank); one ACT instruction may span at most 2 PSUM
banks; sem updates on activations must be attached (then_inc) so they
fire at full retire, not via a following sem_inc.
"""

import numpy as np

NATT, NTEMP, C = 4, 8, 1024
H, W = 256, 256
NPAIR = NTEMP * (NTEMP - 1) // 2
P = 128
N_CORES = 8
STATS_COLS = 4

_nc_cache = None


def _build():
    import concourse.bacc as bacc
    from concourse import mybir

    f32 = mybir.dt.float32
    fp8 = mybir.dt.float8e4
    nc = bacc.Bacc(enable_partition_id=False)
    sb_in = nc.dram_tensor("sb", [P, NTEMP, 2, H], fp8, kind="ExternalInput")
    st_out = nc.dram_tensor("stats", [P, STATS_COLS], f32, kind="ExternalOutput")

    with (
        nc.sbuf_tensor("ht", [P, NTEMP, 2, H], fp8) as ht,
        nc.sbuf_tensor("mmt", [P, H], f32) as mmt,
        nc.sbuf_tensor("mm2", [P, 2, H], f32) as mm2,
        nc.sbuf_tensor("stats_sb", [P, STATS_COLS], f32) as stats,
        nc.sbuf_tensor("scr", [P, 9, H], f32) as scr,
        nc.psum_tensor("ps", [P, NTEMP, H], f32) as ps,
        nc.semaphore("d0") as d0,
        nc.semaphore("d1") as d1,
        nc.semaphore("d2") as d2,
        nc.semaphore("d3") as d3,
        nc.semaphore("osem") as osem,
        nc.semaphore("pesem") as pesem,
        nc.semaphore("vc") as vc,
        nc.semaphore("asem") as asem,
        nc.semaphore("gsem") as gsem,
    ):
        # Input DMAs: two chunks per HWDGE ring so the first pair lands
        # as early as possible while both rings stream in parallel.
        # One semaphore per DMA: a transfer's 16 per-engine increments
        # are not atomic, so chunks must not share a semaphore.
        nc.sync.dma_start(out=ht[:, 0:2], in_=sb_in[:, 0:2]).then_inc(d0, 16)
        nc.sync.dma_start(out=ht[:, 2:4], in_=sb_in[:, 2:4]).then_inc(d1, 16)
        nc.sync.dma_start(out=ht[:, 4:6], in_=sb_in[:, 4:6]).then_inc(d2, 16)
        nc.sync.dma_start(out=ht[:, 6:8], in_=sb_in[:, 6:8]).then_inc(d3, 16)

        # Tensor: one DoubleRow matmul per temp; ps[:, t] is half a PSUM
        # bank, pair p == bank p, so readers of bank p never race the PE
        # (it is writing bank p+1 by the time pesem >= 2p+2).
        waits = {0: (d0, 16), 2: (d1, 16), 4: (d2, 16), 6: (d3, 16)}
        for t in range(NTEMP):
            if t in waits:
                s, v = waits[t]
                nc.tensor.wait_ge(s, v)
            nc.tensor.matmul(
                ps[:, t],
                lhsT=ht[:, t, :, 0:P],
                rhs=ht[:, t],
                start=True,
                stop=True,
                perf_mode=mybir.MatmulPerfMode.DoubleRow,
            ).then_inc(pesem, 1)

        # Vector: M = sum_t A_t rows, folded as each pair completes.
        # DVE may read at most one PSUM operand per instruction, and the
        # engine pipeline gives no same-engine RAW ordering, so the
        # running sum is chained through `vc`.
        # GpSimd (otherwise idle): zero-fill the pad column so the out
        # tile is fully initialized.
        nc.gpsimd.memset(stats[:, 3:4], 0.0).then_inc(gsem, 1)

        nc.vector.wait_ge(pesem, 2)
        nc.vector.tensor_copy(mm2[:, :, :], ps[:, 0:2]).then_inc(vc, 1)
        for p in range(1, 4):
            nc.vector.wait_ge(pesem, 2 * p + 2)
            nc.vector.wait_ge(vc, p)
            nc.vector.tensor_add(
                mm2[:, :, :], mm2[:, :, :], ps[:, 2 * p : 2 * p + 2]
            ).then_inc(vc, 1)

        # Vector tail: fold + |M|^2 while ACT squares quad0.
        nc.vector.wait_ge(vc, 4)
        nc.vector.tensor_add(mmt[:, :], mm2[:, 0], mm2[:, 1]).then_inc(vc, 1)
        nc.vector.wait_ge(vc, 5)
        nc.vector.tensor_mul(scr[:, 8, :], mmt[:, :], mmt[:, :]).then_inc(vc, 1)
        nc.vector.wait_ge(vc, 6)
        nc.vector.reduce_sum(
            stats[:, 1:2], scr[:, 8, :], axis=mybir.AxisListType.X
        ).then_inc(vc, 1)

        # Scalar: Square+accumulate the A blocks in two quads. Empirical
        # device constraints: ACT may only touch PSUM once the PE is
        # completely done (concurrent ACT-read + PE-write hangs the
        # core, regardless of bank); one ACT instruction may span at
        # most 2 PSUM banks; and a second chained ACT is only stable
        # when strictly serialized after all concurrent DVE work.
        # Attached updates fire at full instruction retire (incl. the
        # accumulator read), unlike a following sem_inc.
        nc.scalar.wait_ge(pesem, 8)
        nc.scalar.activation(
            out=scr[:, 0:4],
            in_=ps[:, 0:4],
            func=mybir.ActivationFunctionType.Square,
            accum_out=stats[:, 0:1],
        ).then_inc(asem, 1)
        nc.scalar.wait_ge(vc, 4)
        nc.scalar.activation(
            out=scr[:, 4:8],
            in_=ps[:, 4:8],
            func=mybir.ActivationFunctionType.Square,
            accum_out=stats[:, 2:3],
        ).then_inc(asem, 1)

        # Sync: ship the stats tile once ACT and DVE are done.
        nc.sync.wait_ge(asem, 2)
        nc.sync.wait_ge(vc, 7)
        nc.sync.wait_ge(gsem, 1)
        nc.sync.dma_start(out=st_out[:, :], in_=stats[:, :], single_packet=True).then_inc(osem, 16)
    nc.finalize()
    return nc


last_results = None


def _ensure_ntff_hook():
    import sys
    import types

    try:
        import antenv.axon_hooks  # noqa: F401

        return
    except ImportError:
        pass
    try:
        from trn_agent_boot.trn_boot import _ntff_profile_via_ctypes

        hook = _ntff_profile_via_ctypes("/opt/axon/libaxon_pjrt.so")
    except Exception:
        hook = None
    mod = types.ModuleType("antenv.axon_hooks")
    mod.get_axon_ntff_profile_hook = lambda: hook
    mod.set_axon_ntff_profile_hook = lambda h: None
    sys.modules["antenv.axon_hooks"] = mod


def kernel(attc: np.ndarray, atts: np.ndarray) -> np.ndarray:
    global _nc_cache, last_results
    _ensure_ntff_hook()
    import ml_dtypes
    from concourse.bass_utils import run_bass_kernel_spmd

    if _nc_cache is None:
        _nc_cache = _build()
    nc = _nc_cache

    attc = np.asarray(attc)
    atts = np.asarray(atts)

    in_maps = []
    sb_cache = {}
    for core in range(N_CORES):
        a, m = core // 2, core % 2
        if (a, m) not in sb_cache:
            q = atts[a].astype(ml_dtypes.float8_e4m3fn)  # [8,256,256]
            sb = np.ascontiguousarray(
                q.transpose(2, 0, 1).reshape(P, 2, NTEMP, H).transpose(0, 2, 1, 3)
            )
            sb_cache[(a, 0)] = sb
            sb_cache[(a, 1)] = np.ascontiguousarray(
                np.concatenate([sb[:, :, :, P:], sb[:, :, :, :P]], axis=3)
            )
        in_maps.append({"sb": sb_cache[(a, m)]})

    res = run_bass_kernel_spmd(nc, in_maps, core_ids=list(range(N_CORES)))
    last_results = res
    outs = res.results

    total = 0.0
    for a in range(NATT):
        st0 = outs[2 * a]["stats"].astype(np.float64)
        st1 = outs[2 * a + 1]["stats"].astype(np.float64)
        sumA = st0[:, 0].sum() + st0[:, 2].sum() + st1[:, 0].sum() + st1[:, 2].sum()
        M2 = st0[:, 1].sum() + st1[:, 1].sum()

        S = atts[a].astype(np.float64)  # [8,256,256]
        R = S.sum(0)
        T = (S * S).sum()
        loss_s = (0.5 * (M2 - sumA) - ((R * R).sum() - T) + NPAIR * W) / (W * W)

        c = attc[a].astype(np.float64)  # [8,1024]
        n_t = (c * c).sum(1)
        v = c.sum(0)
        loss_c = (
            0.5 * (n_t.sum() ** 2 - (n_t * n_t).sum())
            - ((v * v).sum() - n_t.sum())
            + NPAIR * C
        ) / (C * C)
        total += loss_s + loss_c

    return np.array(total, dtype=np.float32)


# revision 4
# speedup vs baseline: 1.0270x; 1.0270x over previous
"""AttMatrixCov loss kernel for 8 Trainium2 NeuronCores (raw Bass).

Math (same as the earlier Tile version): with A_t = S_t S_t^T the
pairwise sums collapse to
    sum_{i<j} |S_i^T S_j|_F^2 = 1/2 (|M|_F^2 - sum_t |A_t|_F^2),
    M = sum_t A_t,
plus exact O(N^2) host glue for the R-term and the channel branch.
Sharding: 8 cores = (natt=4) x (H row-block m=0,1); each core loads all
8 temps of S^T as fp8 [128p, 8t, 2g, 256h] and computes its 128-row
block of every A_t and of M.

The device program is hand-scheduled raw Bass (no TileContext - its
entry/exit barriers and scheduling slack cost ~5us here):

  sync:   4x input DMA (2 temps each, pipelined on one HWDGE ring,
          one semaphore per DMA since per-engine sem increments from
          different transfers interleave) -> stats out DMA. No final
          osem wait: the 2KB output lands during the multi-us exit
          sequence, long before the runtime fetches outputs.
  tensor: 8x fp8 DoubleRow matmul, each gated only on its chunk's DMA
          semaphore; runs the stream without interruption.
  vector: M = sum_t A_t folded pair-wise [128,512] as pairs complete
          (max one PSUM operand per DVE op), then fold + |M|^2
          mul/reduce while ACT squares.
  scalar: Square+accumulate the A blocks in two 2-bank quads, then
          quad1 chained behind quad0.
  gpsimd: zero-fills the pad column of the out tile.

Empirical device constraints honored here (each violation hangs the
core): ACT may only touch PSUM after the PE is completely done
(regardless of bank); one ACT instruction may span at most 2 PSUM
banks; sem updates on activations must be attached (then_inc) so they
fire at full retire, not via a following sem_inc.
"""

import numpy as np

NATT, NTEMP, C = 4, 8, 1024
H, W = 256, 256
NPAIR = NTEMP * (NTEMP - 1) // 2
P = 128
N_CORES = 8
STATS_COLS = 4

_nc_cache = None


def _build():
    import concourse.bacc as bacc
    from concourse import mybir

    f32 = mybir.dt.float32
    fp8 = mybir.dt.float8e4
    nc = bacc.Bacc(enable_partition_id=False)
    sb_in = nc.dram_tensor("sb", [P, NTEMP, 2, H], fp8, kind="ExternalInput")
    st_out = nc.dram_tensor("stats", [P, STATS_COLS], f32, kind="ExternalOutput")

    with (
        nc.sbuf_tensor("ht", [P, NTEMP, 2, H], fp8) as ht,
        nc.sbuf_tensor("mmt", [P, H], f32) as mmt,
        nc.sbuf_tensor("mm2", [P, 2, H], f32) as mm2,
        nc.sbuf_tensor("stats_sb", [P, STATS_COLS], f32) as stats,
        nc.sbuf_tensor("scr", [P, 9, H], f32) as scr,
        nc.psum_tensor("ps", [P, NTEMP, H], f32) as ps,
        nc.semaphore("d0") as d0,
        nc.semaphore("d1") as d1,
        nc.semaphore("d2") as d2,
        nc.semaphore("d3") as d3,
        nc.semaphore("osem") as osem,
        nc.semaphore("pesem") as pesem,
        nc.semaphore("vc") as vc,
        nc.semaphore("asem") as asem,
        nc.semaphore("gsem") as gsem,
    ):
        # Input DMAs: two chunks per HWDGE ring so the first pair lands
        # as early as possible while both rings stream in parallel.
        # One semaphore per DMA: a transfer's 16 per-engine increments
        # are not atomic, so chunks must not share a semaphore.
        nc.sync.dma_start(out=ht[:, 0:2], in_=sb_in[:, 0:2]).then_inc(d0, 16)
        nc.scalar.dma_start(out=ht[:, 2:4], in_=sb_in[:, 2:4]).then_inc(d1, 16)
        nc.sync.dma_start(out=ht[:, 4:6], in_=sb_in[:, 4:6]).then_inc(d2, 16)
        nc.scalar.dma_start(out=ht[:, 6:8], in_=sb_in[:, 6:8]).then_inc(d3, 16)

        # Tensor: one DoubleRow matmul per temp; ps[:, t] is half a PSUM
        # bank, pair p == bank p, so readers of bank p never race the PE
        # (it is writing bank p+1 by the time pesem >= 2p+2).
        waits = {0: (d0, 16), 2: (d1, 16), 4: (d2, 16), 6: (d3, 16)}
        for t in range(NTEMP):
            if t in waits:
                s, v = waits[t]
                nc.tensor.wait_ge(s, v)
            nc.tensor.matmul(
                ps[:, t],
                lhsT=ht[:, t, :, 0:P],
                rhs=ht[:, t],
                start=True,
                stop=True,
                perf_mode=mybir.MatmulPerfMode.DoubleRow,
            ).then_inc(pesem, 1)

        # Vector: M = sum_t A_t rows, folded as each pair completes.
        # DVE may read at most one PSUM operand per instruction, and the
        # engine pipeline gives no same-engine RAW ordering, so the
        # running sum is chained through `vc`.
        # GpSimd (otherwise idle): zero-fill the pad column so the out
        # tile is fully initialized.
        nc.gpsimd.memset(stats[:, 3:4], 0.0).then_inc(gsem, 1)

        nc.vector.wait_ge(pesem, 2)
        nc.vector.tensor_copy(mm2[:, :, :], ps[:, 0:2]).then_inc(vc, 1)
        for p in range(1, 4):
            nc.vector.wait_ge(pesem, 2 * p + 2)
            nc.vector.wait_ge(vc, p)
            nc.vector.tensor_add(
                mm2[:, :, :], mm2[:, :, :], ps[:, 2 * p : 2 * p + 2]
            ).then_inc(vc, 1)

        # Vector tail: fold + |M|^2 while ACT squares quad0.
        nc.vector.wait_ge(vc, 4)
        nc.vector.tensor_add(mmt[:, :], mm2[:, 0], mm2[:, 1]).then_inc(vc, 1)
        nc.vector.wait_ge(vc, 5)
        nc.vector.tensor_mul(scr[:, 8, :], mmt[:, :], mmt[:, :]).then_inc(vc, 1)
        nc.vector.wait_ge(vc, 6)
        nc.vector.reduce_sum(
            stats[:, 1:2], scr[:, 8, :], axis=mybir.AxisListType.X
        ).then_inc(vc, 1)

        # Scalar: Square+accumulate the A blocks in two quads. Empirical
        # device constraints: ACT may only touch PSUM once the PE is
        # completely done (concurrent ACT-read + PE-write hangs the
        # core, regardless of bank); one ACT instruction may span at
        # most 2 PSUM banks; and a second chained ACT is only stable
        # when strictly serialized after all concurrent DVE work.
        # Attached updates fire at full instruction retire (incl. the
        # accumulator read), unlike a following sem_inc.
        nc.scalar.wait_ge(pesem, 8)
        nc.scalar.activation(
            out=scr[:, 0:4],
            in_=ps[:, 0:4],
            func=mybir.ActivationFunctionType.Square,
            accum_out=stats[:, 0:1],
        ).then_inc(asem, 1)
        nc.scalar.wait_ge(vc, 4)
        nc.scalar.activation(
            out=scr[:, 4:8],
            in_=ps[:, 4:8],
            func=mybir.ActivationFunctionType.Square,
            accum_out=stats[:, 2:3],
        ).then_inc(asem, 1)

        # Sync: ship the stats tile once ACT and DVE are done.
        nc.sync.wait_ge(asem, 2)
        nc.sync.wait_ge(vc, 7)
        nc.sync.wait_ge(gsem, 1)
        nc.sync.dma_start(out=st_out[:, :], in_=stats[:, :], single_packet=True).then_inc(osem, 16)
    nc.finalize()
    return nc


last_results = None


def _ensure_ntff_hook():
    import sys
    import types

    try:
        import antenv.axon_hooks  # noqa: F401

        return
    except ImportError:
        pass
    try:
        from trn_agent_boot.trn_boot import _ntff_profile_via_ctypes

        hook = _ntff_profile_via_ctypes("/opt/axon/libaxon_pjrt.so")
    except Exception:
        hook = None
    mod = types.ModuleType("antenv.axon_hooks")
    mod.get_axon_ntff_profile_hook = lambda: hook
    mod.set_axon_ntff_profile_hook = lambda h: None
    sys.modules["antenv.axon_hooks"] = mod


def kernel(attc: np.ndarray, atts: np.ndarray) -> np.ndarray:
    global _nc_cache, last_results
    _ensure_ntff_hook()
    import ml_dtypes
    from concourse.bass_utils import run_bass_kernel_spmd

    if _nc_cache is None:
        _nc_cache = _build()
    nc = _nc_cache

    attc = np.asarray(attc)
    atts = np.asarray(atts)

    in_maps = []
    sb_cache = {}
    for core in range(N_CORES):
        a, m = core // 2, core % 2
        if (a, m) not in sb_cache:
            q = atts[a].astype(ml_dtypes.float8_e4m3fn)  # [8,256,256]
            sb = np.ascontiguousarray(
                q.transpose(2, 0, 1).reshape(P, 2, NTEMP, H).transpose(0, 2, 1, 3)
            )
            sb_cache[(a, 0)] = sb
            sb_cache[(a, 1)] = np.ascontiguousarray(
                np.concatenate([sb[:, :, :, P:], sb[:, :, :, :P]], axis=3)
            )
        in_maps.append({"sb": sb_cache[(a, m)]})

    res = run_bass_kernel_spmd(nc, in_maps, core_ids=list(range(N_CORES)))
    last_results = res
    outs = res.results

    total = 0.0
    for a in range(NATT):
        st0 = outs[2 * a]["stats"].astype(np.float64)
        st1 = outs[2 * a + 1]["stats"].astype(np.float64)
        sumA = st0[:, 0].sum() + st0[:, 2].sum() + st1[:, 0].sum() + st1[:, 2].sum()
        M2 = st0[:, 1].sum() + st1[:, 1].sum()

        S = atts[a].astype(np.float64)  # [8,256,256]
        R = S.sum(0)
        T = (S * S).sum()
        loss_s = (0.5 * (M2 - sumA) - ((R * R).sum() - T) + NPAIR * W) / (W * W)

        c = attc[a].astype(np.float64)  # [8,1024]
        n_t = (c * c).sum(1)
        v = c.sum(0)
        loss_c = (
            0.5 * (n_t.sum() ** 2 - (n_t * n_t).sum())
            - ((v * v).sum() - n_t.sum())
            + NPAIR * C
        ) / (C * C)
        total += loss_s + loss_c

    return np.array(total, dtype=np.float32)
